# revision 23
# baseline (speedup 1.0000x reference)
"""BiDAF attention on Trainium2 — data-parallel over batch across 8 NeuronCores.

Reference math (per batch b):
    sim[c,q] = cq[c] + qq[q] + mm[c,q]
      where cq = ctx @ w_c, qq = qn @ w_q, mm = (ctx * w_m) @ qn^T
    a    = softmax_q(qmask ? sim : -inf)          # [C, Q]
    c2q  = a @ qn                                  # [C, D]
    smax = max_q(sim);  b = softmax_c(cmask ? smax : -inf)
    q2c  = b @ ctx  (broadcast over c)             # [C, D]
    g    = [ctx | c2q | ctx*c2q | ctx*q2c]         # [C, 4D]

Kernel design (per core, 8 batches; v3):
  - All device I/O is bf16: HBM-bound problem, and the 2e-2 rel-err budget
    dwarfs bf16's ~4e-3 rounding (measured 7e-3 end to end on HW).
  - The g1 = ctx block is not written by the device (host already has ctx):
    device outputs [c2q | ctx*c2q | ctx*q2c] ([C, 3D] bf16) only.
  - Rank-1 sim terms are HOST-precomputed (they are tiny: B*C + B*Q values)
    and packed into padding columns, removing 10 small matmuls + their
    consumers per batch: ctx row = [data(256) | ones | cq+cmaskadd];
    qn row = [data(256) | qq+qmaskadd | qq | data*w_m (256)].
  - DRAM layouts are PARTITION-MAJOR ([128, BL, row]) so every DMA is one
    contiguous descriptor per partition: the cost of a dma_start on the sync
    sequencer is dominated by descriptor count.
  - sim is computed TRANSPOSED: simT [Q=64 part, C=512 free] = qnTw @ ctxT,
    with ctx^T from 8 PE transposes into ONE [128, 1024] PSUM tile and ONE
    fused copy; qn*w_m arrives pre-scaled from the host and is PE-transposed.
  - softmax_q: expT = exp(psim + (qq+qmaskadd)) straight from PSUM (bias is
    a per-partition scalar); no max-subtraction needed (|logits| <= ~10).
    c2q = expT @ qn in 2 pair-matmuls; denominators come from 4 extra N=1
    matmuls against a ones column (sharing the stationary operand), so ONE
    reciprocal covers all 4 chunks and g2 normalization is fused per pair.
  - max_q path must use the UNMASKED sim including masked columns (the
    reference takes max over everything; using the masked max was measured
    to move the output by 0.12 rel — way out of budget): sim_t = psim + qq,
    4 PE transposes, one fused reduce_max, then
    e = max_exp * exp-style host column: e_col = exp(t + cq + cmaskadd)
    via one tiny add (t + [cq+cmaskadd] col) and one exp.
  - q2c row: ones-column sum inside the e@ctx matmul, one reciprocal, one
    scaled copy, one K=1 broadcast matmul; g4 = ctx * q2c multiplies the
    PSUM broadcast directly.
  - Engine split keeps every engine under the ~2.9 us/batch DMA pace:
    ACT: expT, sim_t, e_col, q2c_row, 2 of 4 g2 chunks.
    DVE: ctxT+qnTw copies, reduce_max, 2 reciprocals, g2 pair, g4.
    Pool: sm_final add, fused g3.
"""

import numpy as np
import ml_dtypes

import concourse.bass as bass
import concourse.bacc as bacc
import concourse.tile as tile
from concourse import mybir
from concourse.masks import make_identity
from concourse.bass_utils import run_bass_kernel_spmd

B, C, Q, D = 64, 512, 64, 256
N_CORES = 8
BL = B // N_CORES  # batches per core

F32 = mybir.dt.float32
BF16 = mybir.dt.bfloat16
NPBF16 = ml_dtypes.bfloat16
AX = mybir.AxisListType.X
EXP = mybir.ActivationFunctionType.Exp
COPY = mybir.ActivationFunctionType.Copy
BIG = 1.0e20

NCC = C // 128  # context row chunks (4)
NDC = D // 128  # hidden-dim chunks (2)
DPC = D + 2     # ctx row: [data(256) | ones | cq+cmaskadd]
DPQ = 2 * D + 4  # qn row: [data | qq+qmaskadd | pad | qq f32x2 | data*w_m]
GW = 3 * D      # device output row width


def _emit(tc, ctx_d, qn_d, g_d, reps=1):
    nc = tc.nc
    with (
        tc.tile_pool(name="consts", bufs=1) as consts,
        tc.tile_pool(name="ct", bufs=4) as ct_pool,
        tc.tile_pool(name="ctxT", bufs=3) as ctxT_pool,
        tc.tile_pool(name="qn", bufs=2) as qn_pool,
        tc.tile_pool(name="sim", bufs=3) as sim_pool,
        tc.tile_pool(name="smalls", bufs=4) as small_pool,
        tc.tile_pool(name="gout", bufs=3) as g_pool,
        tc.tile_pool(name="ptc", bufs=1, space="PSUM") as ptc_pool,
        tc.tile_pool(name="pts", bufs=1, space="PSUM") as pts_pool,
        tc.tile_pool(name="psim", bufs=2, space="PSUM") as psim_pool,
        tc.tile_pool(name="psmall", bufs=1, space="PSUM") as psmall_pool,
        tc.tile_pool(name="pc2q", bufs=2, space="PSUM") as pc2q_pool,
        tc.tile_pool(name="pbc", bufs=1, space="PSUM") as pbc_pool,
    ):
        ident = consts.tile([128, 128], BF16)
        make_identity(nc, ident)
        ones_row = consts.tile([1, 128], BF16)
        nc.vector.memset(ones_row, 1.0)
        ones_col = consts.tile([Q, 1], BF16)
        nc.vector.memset(ones_col, 1.0)

        for rep in range(reps):
          # all 8 batches' question rows in one DMA (one desc per partition)
          qn_all = qn_pool.tile([Q, BL, DPQ], BF16, tag="qn")
          nc.sync.dma_start(
              out=qn_all.rearrange("q b d -> q (b d)"),
              in_=qn_d.rearrange("q b d -> q (b d)"),
          )
          for b in range(BL):
            qn_b = qn_all[:, b, :]
            # ---------- load ctx (1 DMA, 1 desc/partition) ----------
            ct_all = ct_pool.tile([128, NCC, DPC], BF16, tag="ct")
            ctf = ct_all.rearrange("p i d -> p (i d)")
            nc.sync.dma_start(out=ctf[:, : 2 * DPC], in_=ctx_d[:, b, : 2 * DPC])
            nc.sync.dma_start(out=ctf[:, 2 * DPC :], in_=ctx_d[:, b, 2 * DPC :])
            ct = [ct_all[:, i, :] for i in range(NCC)]

            # ---------- question transpose (pre-scaled by w_m on host) ----------
            pts1 = pts_pool.tile([128, 256], BF16, tag="pts")
            ptq = pts1[:, :128]
            for j in range(NDC):
                nc.tensor.transpose(
                    ptq[:, Q * j : Q * (j + 1)],
                    qn_b[:, D + 4 + 128 * j : D + 4 + 128 * (j + 1)],
                    ident[:Q, :Q],
                )
            qnTw = sim_pool.tile([128, 2 * Q], BF16, tag="qnTw")
            nc.vector.tensor_copy(qnTw, ptq)

            # ---------- context transpose: one PSUM tile, one fused copy ----------
            ptc = ptc_pool.tile([128, NDC, C], BF16, tag="ptc")
            for j in range(NDC):
                for i in range(NCC):
                    nc.tensor.transpose(
                        ptc[:, j, 128 * i : 128 * (i + 1)],
                        ct[i][:, 128 * j : 128 * (j + 1)],
                        ident,
                    )
            cT = ctxT_pool.tile([128, NDC, C], BF16, tag="ctxT")
            nc.vector.tensor_copy(cT, ptc)

            # ---------- M1: simT [Q, C] = (qn*w_m) @ ctx^T ----------
            psim = psim_pool.tile([Q, C], F32, tag="psim")
            for j in range(NDC):
                nc.tensor.matmul(
                    psim,
                    qnTw[:, Q * j : Q * (j + 1)],
                    cT[:, j, :],
                    start=(j == 0),
                    stop=(j == NDC - 1),
                )

            # expT = exp(simT + qq + qmaskadd)  [Q, C] straight from PSUM
            expT = sim_pool.tile([Q, C], BF16, tag="expT")
            nc.scalar.activation(expT, psim, EXP, bias=qn_b[:, D : D + 1], scale=1.0)
            # sim_t = simT + qq (UNMASKED; for the exact max path)
            sim_t = sim_pool.tile([Q, C], BF16, tag="simt")
            nc.scalar.add(sim_t, psim, qn_b[:, D + 2 : D + 4].bitcast(F32))

            # ---------- t[c] = max_q sim via PE transpose + fused reduce ----------
            ptm = pts_pool.tile([128, 256], BF16, tag="pts")
            for i in range(NCC):
                nc.tensor.transpose(
                    ptm[:, Q * i : Q * (i + 1)],
                    sim_t[:, 128 * i : 128 * (i + 1)],
                    ident[:Q, :Q],
                )
            t_col = small_pool.tile([128, NCC], F32, tag="tcol")
            nc.vector.reduce_max(
                t_col, ptm.rearrange("p (i q) -> p i q", q=Q), axis=AX
            )
            # e_col = exp(t + cq + cmaskadd): host packed cq+cmaskadd in ctx col
            sm2 = small_pool.tile([128, NCC], F32, tag="sm2")
            nc.gpsimd.tensor_add(sm2, t_col, ct_all[:, :, D + 1])
            e_col = small_pool.tile([128, NCC], BF16, tag="ecol")
            nc.scalar.activation(e_col, sm2, EXP)

            # ---------- q2c numerator + sum: [1, D+1] ----------
            psm = psmall_pool.tile([128, 262], F32, tag="psmall")
            # cols 0..256 = q2c numerator+sum (row 0); 258..261 = c2q denominators
            for i in range(NCC):
                nc.tensor.matmul(
                    psm[:1, 0 : D + 1],
                    e_col[:, i : i + 1],
                    ct[i][:, : D + 1],
                    start=(i == 0),
                    stop=(i == NCC - 1),
                    skip_group_check=True,
                )
            s_rec = small_pool.tile([1, 1], F32, tag="srec")
            nc.vector.reciprocal(s_rec, psm[:1, D : D + 1])
            q2c_row = small_pool.tile([1, D], BF16, tag="q2crow")
            nc.scalar.activation(q2c_row, psm[:1, :D], COPY, scale=s_rec)

            # broadcast q2c over 128 partitions via K=1 ones-matmul
            pbc = pbc_pool.tile([128, D], F32, tag="pbc")
            nc.tensor.matmul(pbc, ones_row, q2c_row, start=True, stop=True)

            # ---------- g4 first: it only needs the q2c path ----------
            g_all = g_pool.tile([128, NCC, GW], BF16, tag="gall")
            g_view = g_d[:, b, :].rearrange("p (i m) -> p i m", i=NCC)
            pbc_sb = small_pool.tile([128, D], BF16, tag="pbcsb")
            nc.vector.tensor_copy(pbc_sb, pbc)
            # real-HW GPSIMD runs 2-input ops at ~2.6 cyc/elem (the cost model
            # says ~1): keep the big muls off Pool
            nc.vector.tensor_mul(
                g_all[:, :, 2 * D : 3 * D],
                ct_all[:, :, :D],
                pbc_sb[:, None, :].broadcast_to([128, NCC, D]),
            )
            nc.sync.dma_start(
                out=g_view[:, :, 2 * D :], in_=g_all[:, :, 2 * D :]
            )

            # ---------- c2q pair matmuls + denominators ----------
            pc2q = []
            for p in range(2):
                pcq = pc2q_pool.tile([128, 2, D], F32, tag="pc2q")
                for k in range(2):
                    i = 2 * p + k
                    lhsT = expT[:, 128 * i : 128 * (i + 1)]
                    nc.tensor.matmul(
                        pcq[:, k, :], lhsT, qn_b[:, :D],
                        start=True, stop=True, skip_group_check=True,
                    )
                    nc.tensor.matmul(
                        psm[:, 258 + i : 259 + i], lhsT, ones_col,
                        start=True, stop=True, skip_group_check=True,
                    )
                pc2q.append(pcq)
            den_r = small_pool.tile([128, NCC], F32, tag="denr")
            nc.vector.reciprocal(den_r, psm[:, 258:262])

            # ---------- g2 | g3: [c2q | ctx*c2q] ----------
            # g2 = c2q normalized: chunks 0,1 on ACT; pair 1 fused on DVE
            for i in range(2):
                nc.scalar.activation(
                    g_all[:, i, 0:D], pc2q[0][:, i, :], COPY,
                    scale=den_r[:, i : i + 1],
                )
            nc.vector.tensor_mul(
                g_all[:, 2:4, 0:D],
                pc2q[1],
                den_r[:, 2:4, None].broadcast_to([128, 2, D]),
            )
            # g3 = ctx * c2q — split: pair 0 on Pool, pair 1 on DVE
            nc.gpsimd.tensor_mul(
                g_all[:, :2, D : 2 * D], ct_all[:, :2, :D], g_all[:, :2, 0:D]
            )
            nc.vector.tensor_mul(
                g_all[:, 2:, D : 2 * D], ct_all[:, 2:, :D], g_all[:, 2:, 0:D]
            )
            nc.sync.dma_start(
                out=g_view[:, :, : 2 * D], in_=g_all[:, :, : 2 * D]
            )


def _emit_dmaonly(tc, ctx_d, qn_d, g_d, reps=1):
    """Experiment: just the DMA traffic of the real kernel, no compute."""
    nc = tc.nc
    with (
        tc.tile_pool(name="ct", bufs=4) as ct_pool,
        tc.tile_pool(name="qn", bufs=2) as qn_pool,
        tc.tile_pool(name="gout", bufs=3) as g_pool,
    ):
        for rep in range(reps):
            qn_all = qn_pool.tile([Q, BL, DPQ], BF16, tag="qn")
            nc.sync.dma_start(out=qn_all, in_=qn_d[:, :, :])
            for b in range(BL):
                ct_all = ct_pool.tile([128, NCC, DPC], BF16, tag="ct")
                nc.sync.dma_start(
                    out=ct_all,
                    in_=ctx_d[:, b, :].rearrange("p (i d) -> p i d", i=NCC),
                )
                g_all = g_pool.tile([128, NCC, GW], BF16, tag="gall")
                nc.vector.memset(g_all[:, :, 0:2], 1.0)
                nc.sync.dma_start(
                    out=g_d[:, b, :].rearrange("p (i m) -> p i m", i=NCC),
                    in_=g_all,
                )


def build_module(compile=True, reps=1, variant="full"):
    nc = bacc.Bacc(trn_type="TRN2")
    ctx_d = nc.dram_tensor("context", [128, BL, NCC * DPC], BF16, kind="ExternalInput")
    qn_d = nc.dram_tensor("question", [Q, BL, DPQ], BF16, kind="ExternalInput")
    g_d = nc.dram_tensor("g", [128, BL, NCC * GW], BF16, kind="ExternalOutput")
    emit = {"full": _emit, "dmaonly": _emit_dmaonly}[variant]
    with tile.TileContext(nc) as tc:
        emit(tc, ctx_d, qn_d, g_d, reps=reps)
    if compile:
        nc.compile()
    return nc


_NC_CACHE = None


def _get_module():
    global _NC_CACHE
    if _NC_CACHE is None:
        _NC_CACHE = build_module()
    return _NC_CACHE


def make_in_maps(context, question, context_mask, question_mask, w):
    context = np.asarray(context, dtype=np.float32)
    question = np.asarray(question, dtype=np.float32)
    w = np.asarray(w, dtype=np.float32)
    w_c, w_q, w_m = w[:D], w[D : 2 * D], w[2 * D :]
    cmadd = (np.asarray(context_mask, dtype=np.float32) - 1.0) * BIG
    qmadd = (np.asarray(question_mask, dtype=np.float32) - 1.0) * BIG
    cq = context @ w_c  # [B, C]
    qq = question @ w_q  # [B, Q]

    ctx_p = np.zeros((B, C, DPC), dtype=np.float32)
    ctx_p[:, :, :D] = context
    ctx_p[:, :, D] = 1.0
    ctx_p[:, :, D + 1] = cq + cmadd
    ctx_p = ctx_p.astype(NPBF16)
    # partition-major: [128, B, NCC*DPC]
    ctx_p = ctx_p.reshape(B, NCC, 128, DPC).transpose(2, 0, 1, 3)
    ctx_p = ctx_p.reshape(128, B, NCC * DPC)

    qn_p = np.zeros((B, Q, DPQ), dtype=np.float32)
    qn_p[:, :, :D] = question
    qn_p[:, :, D] = qq + qmadd
    qn_p[:, :, D + 4 :] = question * w_m
    qn_p = qn_p.astype(NPBF16)
    # qq as raw f32 occupying two bf16 slots (4-byte aligned at col D+2)
    qn_p.view(np.uint16)[:, :, D + 2 : D + 4] = (
        qq.astype(np.float32).view(np.uint32).astype(np.uint32)[..., None]
        .view(np.uint16).reshape(B, Q, 2)
    )
    qn_p = qn_p.transpose(1, 0, 2)  # [Q, B, DPQ]

    in_maps = []
    for k in range(N_CORES):
        sl = slice(k * BL, (k + 1) * BL)
        in_maps.append(
            {
                "context": np.ascontiguousarray(ctx_p[:, sl]),
                "question": np.ascontiguousarray(qn_p[:, sl]),
            }
        )
    return in_maps


def kernel(context, question, context_mask, question_mask, w):
    nc = _get_module()
    in_maps = make_in_maps(context, question, context_mask, question_mask, w)
    res = run_bass_kernel_spmd(nc, in_maps, list(range(N_CORES)))
    out = np.empty((B, C, 4 * D), dtype=np.float32)
    out[:, :, :D] = np.asarray(context, dtype=np.float32)
    for k in range(N_CORES):
        gk = np.asarray(res.results[k]["g"])  # [128, BL, NCC*GW] bf16
        gk = gk.reshape(128, BL, NCC, GW).transpose(1, 2, 0, 3).reshape(BL, C, GW)
        out[k * BL : (k + 1) * BL, :, D:] = gk.astype(np.float32)
    return out


# revision 30
# speedup vs baseline: 1.3079x; 1.3079x over previous
"""BiDAF attention on Trainium2 — data-parallel over batch across 8 NeuronCores.

Reference math (per batch b):
    sim[c,q] = cq[c] + qq[q] + mm[c,q]
      where cq = ctx @ w_c, qq = qn @ w_q, mm = (ctx * w_m) @ qn^T
    a    = softmax_q(qmask ? sim : -inf)          # [C, Q]
    c2q  = a @ qn                                  # [C, D]
    smax = max_q(sim);  b = softmax_c(cmask ? smax : -inf)
    q2c  = b @ ctx  (broadcast over c)             # [C, D]
    g    = [ctx | c2q | ctx*c2q | ctx*q2c]         # [C, 4D]

Kernel design (per core, 8 batches; v3):
  - All device I/O is bf16: HBM-bound problem, and the 2e-2 rel-err budget
    dwarfs bf16's ~4e-3 rounding (measured 7e-3 end to end on HW).
  - The g1 = ctx block is not written by the device (host already has ctx):
    device outputs [c2q | ctx*c2q | ctx*q2c] ([C, 3D] bf16) only.
  - Rank-1 sim terms are HOST-precomputed (they are tiny: B*C + B*Q values)
    and packed into padding columns, removing 10 small matmuls + their
    consumers per batch: ctx row = [data(256) | ones | cq+cmaskadd];
    qn row = [data(256) | qq+qmaskadd | qq | data*w_m (256)].
  - DRAM layouts are PARTITION-MAJOR ([128, BL, row]) so every DMA is one
    contiguous descriptor per partition: the cost of a dma_start on the sync
    sequencer is dominated by descriptor count.
  - sim is computed TRANSPOSED: simT [Q=64 part, C=512 free] = qnTw @ ctxT,
    with ctx^T from 8 PE transposes into ONE [128, 1024] PSUM tile and ONE
    fused copy; qn*w_m arrives pre-scaled from the host and is PE-transposed.
  - softmax_q: expT = exp(psim + (qq+qmaskadd)) straight from PSUM (bias is
    a per-partition scalar); no max-subtraction needed (|logits| <= ~10).
    c2q = expT @ qn in 2 pair-matmuls; denominators come from 4 extra N=1
    matmuls against a ones column (sharing the stationary operand), so ONE
    reciprocal covers all 4 chunks and g2 normalization is fused per pair.
  - max_q path must use the UNMASKED sim including masked columns (the
    reference takes max over everything; using the masked max was measured
    to move the output by 0.12 rel — way out of budget): sim_t = psim + qq,
    4 PE transposes, one fused reduce_max, then
    e = max_exp * exp-style host column: e_col = exp(t + cq + cmaskadd)
    via one tiny add (t + [cq+cmaskadd] col) and one exp.
  - q2c row: ones-column sum inside the e@ctx matmul, one reciprocal, one
    scaled copy, one K=1 broadcast matmul; g4 = ctx * q2c multiplies the
    PSUM broadcast directly.
  - Engine split keeps every engine under the ~2.9 us/batch DMA pace:
    ACT: expT, sim_t, e_col, q2c_row, 2 of 4 g2 chunks.
    DVE: ctxT+qnTw copies, reduce_max, 2 reciprocals, g2 pair, g4.
    Pool: sm_final add, fused g3.
"""

import numpy as np
import ml_dtypes

import concourse.bass as bass
import concourse.bacc as bacc
import concourse.tile as tile
from concourse import mybir
from concourse.masks import make_identity
from concourse.bass_utils import run_bass_kernel_spmd

B, C, Q, D = 64, 512, 64, 256
N_CORES = 8
BL = B // N_CORES  # batches per core

F32 = mybir.dt.float32
BF16 = mybir.dt.bfloat16
NPBF16 = ml_dtypes.bfloat16
AX = mybir.AxisListType.X
EXP = mybir.ActivationFunctionType.Exp
COPY = mybir.ActivationFunctionType.Copy
BIG = 1.0e20

NCC = C // 128  # context row chunks (4)
NDC = D // 128  # hidden-dim chunks (2)
DPC = D + 2     # ctx row: [data(256) | ones | cq+cmaskadd]
DPQ = 2 * D + 4  # qn row: [data | qq+qmaskadd | pad | qq f32x2 | data*w_m]
GW = 3 * D      # device output row width

# runtime-tunable engine assignments (A/B experiments); values: "dve"|"pool"
KNOBS = {"g4": "dve", "g3p0": "dve", "stage": "full",
         "psim_bufs": 2, "psmall_bufs": 1, "pbc_bufs": 1, "pc2q_bufs": 2,
         "sm2eng": "pool", "q2crow": "act", "pts_bufs": 1}


def _emit(tc, ctx_d, qn_d, g_d, reps=1):
    """5-stage software pipeline over batches.

    ACT/DVE/Pool are strict in-order FIFO engines: with batch-sequential
    emission, each engine stream serializes the whole ~12-hop cross-engine
    chain per batch (~6 us/batch measured vs ~3 us/batch of engine work).
    Staged emission S1(b) S2(b-1) S3(b-2) S4(b-3) S5(b-4) makes every op's
    inputs come from an earlier iteration, so the FIFOs never block on
    same-batch chains.

      S1: ct DMA, qn/ctx PE transposes, qnTw+cT copies (DVE)
      S2: sim matmuls (PE), expT+sim_t (ACT)
      S3: max transposes (PE), reduce_max+sm2 (DVE), e_col (ACT)
      S4: q2c matmul (PE), s_rec (DVE), q2c_row (ACT), pbc (PE),
          pbc_sb+g4 (DVE), g4 store
      S5: c2q+den matmuls (PE), den_r (DVE), g2 (ACT+DVE), g3 (Pool+DVE),
          g2|g3 store
    """
    nc = tc.nc
    with (
        tc.tile_pool(name="consts", bufs=1) as consts,
        tc.tile_pool(name="ct", bufs=6) as ct_pool,
        tc.tile_pool(name="ctxT", bufs=3) as ctxT_pool,
        tc.tile_pool(name="qn", bufs=2) as qn_pool,
        tc.tile_pool(name="sim", bufs=5) as sim_pool,
        tc.tile_pool(name="smalls", bufs=4) as small_pool,
        tc.tile_pool(name="gout", bufs=3) as g_pool,
        tc.tile_pool(name="ptc", bufs=1, space="PSUM") as ptc_pool,
        tc.tile_pool(name="pts", bufs=2, space="PSUM") as pts_pool,
        tc.tile_pool(name="psim", bufs=1, space="PSUM") as psim_pool,
        tc.tile_pool(name="psmall", bufs=2, space="PSUM") as psmall_pool,
        tc.tile_pool(name="pc2q", bufs=1, space="PSUM") as pc2q_pool,
        tc.tile_pool(name="pbc", bufs=1, space="PSUM") as pbc_pool,
    ):
        ident = consts.tile([128, 128], BF16)
        make_identity(nc, ident)
        ones_row = consts.tile([1, 128], BF16)
        nc.vector.memset(ones_row, 1.0)
        ones_col = consts.tile([Q, 1], BF16)
        nc.vector.memset(ones_col, 1.0)

        for rep in range(reps):
          qn_all = qn_pool.tile([Q, BL, DPQ], BF16, tag="qn")
          nc.sync.dma_start(
              out=qn_all.rearrange("q b d -> q (b d)"),
              in_=qn_d.rearrange("q b d -> q (b d)"),
          )
          st = [dict() for _ in range(BL)]

          def S1(b):
            s = st[b]
            s["qn_b"] = qn_all[:, b, :]
            ct_all = ct_pool.tile([128, NCC, DPC], BF16, tag="ct")
            nc.sync.dma_start(
                out=ct_all.rearrange("p i d -> p (i d)"), in_=ctx_d[:, b, :]
            )
            s["ct"] = ct_all
            pts1 = pts_pool.tile([128, 256], BF16, tag="pts")
            for j in range(NDC):
                nc.tensor.transpose(
                    pts1[:, Q * j : Q * (j + 1)],
                    s["qn_b"][:, D + 4 + 128 * j : D + 4 + 128 * (j + 1)],
                    ident[:Q, :Q],
                )
            qnTw = sim_pool.tile([128, 2 * Q], BF16, tag="qnTw")
            nc.vector.tensor_copy(qnTw, pts1[:, : 2 * Q])
            s["qnTw"] = qnTw
            ptc = ptc_pool.tile([128, NDC, C], BF16, tag="ptc")
            for j in range(NDC):
                for i in range(NCC):
                    nc.tensor.transpose(
                        ptc[:, j, 128 * i : 128 * (i + 1)],
                        ct_all[:, i, 128 * j : 128 * (j + 1)],
                        ident,
                    )
            cT = ctxT_pool.tile([128, NDC, C], BF16, tag="ctxT")
            nc.vector.tensor_copy(cT, ptc)
            s["cT"] = cT

          def S2(b):
            s = st[b]
            psim = psim_pool.tile([Q, C], F32, tag="psim")
            for j in range(NDC):
                nc.tensor.matmul(
                    psim,
                    s["qnTw"][:, Q * j : Q * (j + 1)],
                    s["cT"][:, j, :],
                    start=(j == 0),
                    stop=(j == NDC - 1),
                )
            expT = sim_pool.tile([Q, C], BF16, tag="expT")
            nc.scalar.activation(
                expT, psim, EXP, bias=s["qn_b"][:, D : D + 1], scale=1.0
            )
            s["expT"] = expT
            sim_t = sim_pool.tile([Q, C], BF16, tag="simt")
            nc.scalar.add(sim_t, psim, s["qn_b"][:, D + 2 : D + 4].bitcast(F32))
            s["sim_t"] = sim_t

          def S3(b):
            s = st[b]
            ptm = pts_pool.tile([128, 256], BF16, tag="pts")
            for i in range(NCC):
                nc.tensor.transpose(
                    ptm[:, Q * i : Q * (i + 1)],
                    s["sim_t"][:, 128 * i : 128 * (i + 1)],
                    ident[:Q, :Q],
                )
            t_col = small_pool.tile([128, NCC], F32, tag="tcol")
            nc.vector.reduce_max(
                t_col, ptm.rearrange("p (i q) -> p i q", q=Q), axis=AX
            )
            sm2 = small_pool.tile([128, NCC], F32, tag="sm2")
            nc.vector.tensor_add(sm2, t_col, s["ct"][:, :, D + 1])
            e_col = small_pool.tile([128, NCC], BF16, tag="ecol")
            nc.scalar.activation(e_col, sm2, EXP)
            s["e_col"] = e_col

          def S4(b):
            s = st[b]
            psm = psmall_pool.tile([128, 262], F32, tag="psmall")
            s["psm"] = psm
            for i in range(NCC):
                nc.tensor.matmul(
                    psm[:1, 0 : D + 1],
                    s["e_col"][:, i : i + 1],
                    s["ct"][:, i, : D + 1],
                    start=(i == 0),
                    stop=(i == NCC - 1),
                    skip_group_check=True,
                )
            s_rec = small_pool.tile([1, 1], F32, tag="srec")
            nc.vector.reciprocal(s_rec, psm[:1, D : D + 1])
            q2c_row = small_pool.tile([1, D], BF16, tag="q2crow")
            nc.scalar.activation(q2c_row, psm[:1, :D], COPY, scale=s_rec)
            pbc = pbc_pool.tile([128, D], F32, tag="pbc")
            nc.tensor.matmul(pbc, ones_row, q2c_row, start=True, stop=True)
            pbc_sb = small_pool.tile([128, D], BF16, tag="pbcsb")
            nc.vector.tensor_copy(pbc_sb, pbc)
            g_all = g_pool.tile([128, NCC, GW], BF16, tag="gall")
            s["g_all"] = g_all
            g_view = g_d[:, b, :].rearrange("p (i m) -> p i m", i=NCC)
            s["g_view"] = g_view
            nc.vector.tensor_mul(
                g_all[:, :, 2 * D : 3 * D],
                s["ct"][:, :, :D],
                pbc_sb[:, None, :].broadcast_to([128, NCC, D]),
            )
            nc.sync.dma_start(
                out=g_view[:, :, 2 * D :], in_=g_all[:, :, 2 * D :]
            )

          def S5(b):
            s = st[b]
            g_all, psm = s["g_all"], s["psm"]
            pc2q = []
            for p in range(2):
                pcq = pc2q_pool.tile([128, 2, D], F32, tag="pc2q")
                for k in range(2):
                    i = 2 * p + k
                    lhsT = s["expT"][:, 128 * i : 128 * (i + 1)]
                    nc.tensor.matmul(
                        pcq[:, k, :], lhsT, s["qn_b"][:, :D],
                        start=True, stop=True, skip_group_check=True,
                    )
                    nc.tensor.matmul(
                        psm[:, 258 + i : 259 + i], lhsT, ones_col,
                        start=True, stop=True, skip_group_check=True,
                    )
                pc2q.append(pcq)
            den_r = small_pool.tile([128, NCC], F32, tag="denr")
            nc.vector.reciprocal(den_r, psm[:, 258:262])
            for i in range(2):
                nc.scalar.activation(
                    g_all[:, i, 0:D], pc2q[0][:, i, :], COPY,
                    scale=den_r[:, i : i + 1],
                )
            nc.vector.tensor_mul(
                g_all[:, 2:4, 0:D],
                pc2q[1],
                den_r[:, 2:4, None].broadcast_to([128, 2, D]),
            )
            nc.vector.tensor_mul(
                g_all[:, 2:, D : 2 * D], s["ct"][:, 2:, :D], g_all[:, 2:, 0:D]
            )
            g3eng = nc.gpsimd if KNOBS["g3p0"] == "pool" else nc.vector
            g3eng.tensor_mul(
                g_all[:, :2, D : 2 * D], s["ct"][:, :2, :D], g_all[:, :2, 0:D]
            )
            nc.sync.dma_start(
                out=s["g_view"][:, :, : 2 * D], in_=g_all[:, :, : 2 * D]
            )

          stages = [S1, S2, S3, S4, S5]
          for i in range(BL + len(stages) - 1):
              for k, stage in enumerate(stages):
                  if 0 <= i - k < BL:
                      stage(i - k)


def _emit_dmaonly(tc, ctx_d, qn_d, g_d, reps=1):
    """Experiment: just the DMA traffic of the real kernel, no compute."""
    nc = tc.nc
    with (
        tc.tile_pool(name="ct", bufs=4) as ct_pool,
        tc.tile_pool(name="qn", bufs=2) as qn_pool,
        tc.tile_pool(name="gout", bufs=3) as g_pool,
    ):
        for rep in range(reps):
            qn_all = qn_pool.tile([Q, BL, DPQ], BF16, tag="qn")
            nc.sync.dma_start(out=qn_all, in_=qn_d[:, :, :])
            for b in range(BL):
                ct_all = ct_pool.tile([128, NCC, DPC], BF16, tag="ct")
                nc.sync.dma_start(
                    out=ct_all,
                    in_=ctx_d[:, b, :].rearrange("p (i d) -> p i d", i=NCC),
                )
                g_all = g_pool.tile([128, NCC, GW], BF16, tag="gall")
                nc.vector.memset(g_all[:, :, 0:2], 1.0)
                nc.sync.dma_start(
                    out=g_d[:, b, :].rearrange("p (i m) -> p i m", i=NCC),
                    in_=g_all,
                )




def _emit_probe(tc, ctx_d, qn_d, g_d, reps=1):
    """Per-op rate probe: 64x one op per rep, no DMA in the loop."""
    nc = tc.nc
    op = KNOBS.get("probe_op", "act_copy")
    with (
        tc.tile_pool(name="src", bufs=1) as src_pool,
        tc.tile_pool(name="dst", bufs=2) as dst_pool,
    ):
        a = src_pool.tile([128, 4, D], BF16)
        nc.vector.memset(a, 1.0)
        b = src_pool.tile([128, 4, D], BF16)
        nc.vector.memset(b, 0.5)
        af = src_pool.tile([128, 4, D], F32)
        nc.vector.memset(af, 1.0)
        scal = src_pool.tile([128, 1], F32)
        nc.vector.memset(scal, 0.25)
        row = src_pool.tile([128, D], BF16)
        nc.vector.memset(row, 2.0)
        out_t = src_pool.tile([128, 16], BF16)
        for rep in range(reps):
            for k in range(64):
                o = dst_pool.tile([128, 4, D], BF16, tag="o")
                if op == "act_copy":
                    nc.scalar.activation(
                        o[:, 0, :], a[:, k % 4, :], COPY, scale=scal
                    )
                elif op == "act_exp":
                    nc.scalar.activation(
                        o.rearrange("p i d -> p (i d)")[:Q, :C],
                        a.rearrange("p i d -> p (i d)")[:Q, :C], EXP,
                    )
                elif op == "dve_mul_pair":
                    nc.vector.tensor_mul(o[:, :2, :], a[:, :2, :], b[:, :2, :])
                elif op == "dve_mul_bcast":
                    nc.vector.tensor_mul(
                        o, a, row[:, None, :].broadcast_to([128, 4, D])
                    )
                elif op == "dve_copy_1024":
                    nc.vector.tensor_copy(
                        o.rearrange("p i d -> p (i d)"),
                        a.rearrange("p i d -> p (i d)"),
                    )
                elif op == "dve_mul_f32":
                    nc.vector.tensor_mul(o[:, :2, :], af[:, :2, :], b[:, :2, :])
                elif op == "pool_mul_pair":
                    nc.gpsimd.tensor_mul(o[:, :2, :], a[:, :2, :], b[:, :2, :])
                else:
                    raise ValueError(op)
            # keep a data dependency so reps chain
            nc.vector.tensor_copy(out_t[:, 0:4], o[:, 0, 0:4])
        nc.sync.dma_start(out=g_d[:, 0, 0:16], in_=out_t)


def build_module(compile=True, reps=1, variant="full"):
    nc = bacc.Bacc(trn_type="TRN2")
    ctx_d = nc.dram_tensor("context", [128, BL, NCC * DPC], BF16, kind="ExternalInput")
    qn_d = nc.dram_tensor("question", [Q, BL, DPQ], BF16, kind="ExternalInput")
    g_d = nc.dram_tensor("g", [128, BL, NCC * GW], BF16, kind="ExternalOutput")
    emit = {"full": _emit, "dmaonly": _emit_dmaonly, "probe": _emit_probe}[variant]
    with tile.TileContext(nc) as tc:
        emit(tc, ctx_d, qn_d, g_d, reps=reps)
    if compile:
        nc.compile()
    return nc


_NC_CACHE = None


def _get_module():
    global _NC_CACHE
    if _NC_CACHE is None:
        _NC_CACHE = build_module()
    return _NC_CACHE


def make_in_maps(context, question, context_mask, question_mask, w):
    context = np.asarray(context, dtype=np.float32)
    question = np.asarray(question, dtype=np.float32)
    w = np.asarray(w, dtype=np.float32)
    w_c, w_q, w_m = w[:D], w[D : 2 * D], w[2 * D :]
    cmadd = (np.asarray(context_mask, dtype=np.float32) - 1.0) * BIG
    qmadd = (np.asarray(question_mask, dtype=np.float32) - 1.0) * BIG
    cq = context @ w_c  # [B, C]
    qq = question @ w_q  # [B, Q]

    ctx_p = np.zeros((B, C, DPC), dtype=np.float32)
    ctx_p[:, :, :D] = context
    ctx_p[:, :, D] = 1.0
    ctx_p[:, :, D + 1] = cq + cmadd
    ctx_p = ctx_p.astype(NPBF16)
    # partition-major: [128, B, NCC*DPC]
    ctx_p = ctx_p.reshape(B, NCC, 128, DPC).transpose(2, 0, 1, 3)
    ctx_p = ctx_p.reshape(128, B, NCC * DPC)

    qn_p = np.zeros((B, Q, DPQ), dtype=np.float32)
    qn_p[:, :, :D] = question
    qn_p[:, :, D] = qq + qmadd
    qn_p[:, :, D + 4 :] = question * w_m
    qn_p = qn_p.astype(NPBF16)
    # qq as raw f32 occupying two bf16 slots (4-byte aligned at col D+2)
    qn_p.view(np.uint16)[:, :, D + 2 : D + 4] = (
        qq.astype(np.float32).view(np.uint32).astype(np.uint32)[..., None]
        .view(np.uint16).reshape(B, Q, 2)
    )
    qn_p = qn_p.transpose(1, 0, 2)  # [Q, B, DPQ]

    in_maps = []
    for k in range(N_CORES):
        sl = slice(k * BL, (k + 1) * BL)
        in_maps.append(
            {
                "context": np.ascontiguousarray(ctx_p[:, sl]),
                "question": np.ascontiguousarray(qn_p[:, sl]),
            }
        )
    return in_maps


def kernel(context, question, context_mask, question_mask, w):
    nc = _get_module()
    in_maps = make_in_maps(context, question, context_mask, question_mask, w)
    res = run_bass_kernel_spmd(nc, in_maps, list(range(N_CORES)))
    out = np.empty((B, C, 4 * D), dtype=np.float32)
    out[:, :, :D] = np.asarray(context, dtype=np.float32)
    for k in range(N_CORES):
        gk = np.asarray(res.results[k]["g"])  # [128, BL, NCC*GW] bf16
        gk = gk.reshape(128, BL, NCC, GW).transpose(1, 2, 0, 3).reshape(BL, C, GW)
        out[k * BL : (k + 1) * BL, :, D:] = gk.astype(np.float32)
    return out


# revision 31
# speedup vs baseline: 1.5080x; 1.1530x over previous
"""BiDAF attention on Trainium2 — data-parallel over batch across 8 NeuronCores.

Reference math (per batch b):
    sim[c,q] = cq[c] + qq[q] + mm[c,q]
      where cq = ctx @ w_c, qq = qn @ w_q, mm = (ctx * w_m) @ qn^T
    a    = softmax_q(qmask ? sim : -inf)          # [C, Q]
    c2q  = a @ qn                                  # [C, D]
    smax = max_q(sim);  b = softmax_c(cmask ? smax : -inf)
    q2c  = b @ ctx  (broadcast over c)             # [C, D]
    g    = [ctx | c2q | ctx*c2q | ctx*q2c]         # [C, 4D]

Kernel design (per core, 8 batches; v3):
  - All device I/O is bf16: HBM-bound problem, and the 2e-2 rel-err budget
    dwarfs bf16's ~4e-3 rounding (measured 7e-3 end to end on HW).
  - The g1 = ctx block is not written by the device (host already has ctx):
    device outputs [c2q | ctx*c2q | ctx*q2c] ([C, 3D] bf16) only.
  - Rank-1 sim terms are HOST-precomputed (they are tiny: B*C + B*Q values)
    and packed into padding columns, removing 10 small matmuls + their
    consumers per batch: ctx row = [data(256) | ones | cq+cmaskadd];
    qn row = [data(256) | qq+qmaskadd | qq | data*w_m (256)].
  - DRAM layouts are PARTITION-MAJOR ([128, BL, row]) so every DMA is one
    contiguous descriptor per partition: the cost of a dma_start on the sync
    sequencer is dominated by descriptor count.
  - sim is computed TRANSPOSED: simT [Q=64 part, C=512 free] = qnTw @ ctxT,
    with ctx^T from 8 PE transposes into ONE [128, 1024] PSUM tile and ONE
    fused copy; qn*w_m arrives pre-scaled from the host and is PE-transposed.
  - softmax_q: expT = exp(psim + (qq+qmaskadd)) straight from PSUM (bias is
    a per-partition scalar); no max-subtraction needed (|logits| <= ~10).
    c2q = expT @ qn in 2 pair-matmuls; denominators come from 4 extra N=1
    matmuls against a ones column (sharing the stationary operand), so ONE
    reciprocal covers all 4 chunks and g2 normalization is fused per pair.
  - max_q path must use the UNMASKED sim including masked columns (the
    reference takes max over everything; using the masked max was measured
    to move the output by 0.12 rel — way out of budget): sim_t = psim + qq,
    4 PE transposes, one fused reduce_max, then
    e = max_exp * exp-style host column: e_col = exp(t + cq + cmaskadd)
    via one tiny add (t + [cq+cmaskadd] col) and one exp.
  - q2c row: ones-column sum inside the e@ctx matmul, one reciprocal, one
    scaled copy, one K=1 broadcast matmul; g4 = ctx * q2c multiplies the
    PSUM broadcast directly.
  - Engine split keeps every engine under the ~2.9 us/batch DMA pace:
    ACT: expT, sim_t, e_col, q2c_row, 2 of 4 g2 chunks.
    DVE: ctxT+qnTw copies, reduce_max, 2 reciprocals, g2 pair, g4.
    Pool: sm_final add, fused g3.
"""

import numpy as np
import ml_dtypes

import concourse.bass as bass
import concourse.bacc as bacc
import concourse.tile as tile
from concourse import mybir
from concourse.masks import make_identity
from concourse.bass_utils import run_bass_kernel_spmd

B, C, Q, D = 64, 512, 64, 256
N_CORES = 8
BL = B // N_CORES  # batches per core

F32 = mybir.dt.float32
BF16 = mybir.dt.bfloat16
NPBF16 = ml_dtypes.bfloat16
AX = mybir.AxisListType.X
EXP = mybir.ActivationFunctionType.Exp
COPY = mybir.ActivationFunctionType.Copy
BIG = 1.0e20

NCC = C // 128  # context row chunks (4)
NDC = D // 128  # hidden-dim chunks (2)
DPC = D + 2     # ctx row: [data(256) | ones | cq+cmaskadd]
DPQ = 2 * D + 4  # qn row: [data | qq+qmaskadd | pad | qq f32x2 | data*w_m]
GW = 3 * D      # device output row width

# runtime-tunable engine assignments (A/B experiments); values: "dve"|"pool"
KNOBS = {"g4": "dve", "g3p0": "dve", "stage": "full",
         "psim_bufs": 2, "psmall_bufs": 1, "pbc_bufs": 1, "pc2q_bufs": 2,
         "sm2eng": "pool", "q2crow": "act", "pts_bufs": 1}


def _emit(tc, ctx_d, qn_d, g_d, reps=1):
    """5-stage software pipeline over batches.

    ACT/DVE/Pool are strict in-order FIFO engines: with batch-sequential
    emission, each engine stream serializes the whole ~12-hop cross-engine
    chain per batch (~6 us/batch measured vs ~3 us/batch of engine work).
    Staged emission S1(b) S2(b-1) S3(b-2) S4(b-3) S5(b-4) makes every op's
    inputs come from an earlier iteration, so the FIFOs never block on
    same-batch chains.

      S1: ct DMA, qn/ctx PE transposes, qnTw+cT copies (DVE)
      S2: sim matmuls (PE), expT+sim_t (ACT)
      S3: max transposes (PE), reduce_max+sm2 (DVE), e_col (ACT)
      S4: q2c matmul (PE), s_rec (DVE), q2c_row (ACT), pbc (PE),
          pbc_sb+g4 (DVE), g4 store
      S5: c2q+den matmuls (PE), den_r (DVE), g2 (ACT+DVE), g3 (Pool+DVE),
          g2|g3 store
    """
    nc = tc.nc
    with (
        tc.tile_pool(name="consts", bufs=1) as consts,
        tc.tile_pool(name="ct", bufs=6) as ct_pool,
        tc.tile_pool(name="ctxT", bufs=3) as ctxT_pool,
        tc.tile_pool(name="qn", bufs=2) as qn_pool,
        tc.tile_pool(name="sim", bufs=5) as sim_pool,
        tc.tile_pool(name="smalls", bufs=4) as small_pool,
        tc.tile_pool(name="gout", bufs=3) as g_pool,
        tc.tile_pool(name="ptc", bufs=1, space="PSUM") as ptc_pool,
        tc.tile_pool(name="pts", bufs=2, space="PSUM") as pts_pool,
        tc.tile_pool(name="psim", bufs=1, space="PSUM") as psim_pool,
        tc.tile_pool(name="psmall", bufs=2, space="PSUM") as psmall_pool,
        tc.tile_pool(name="pc2q", bufs=1, space="PSUM") as pc2q_pool,
        tc.tile_pool(name="pbc", bufs=1, space="PSUM") as pbc_pool,
    ):
        ident = consts.tile([128, 128], BF16)
        make_identity(nc, ident)
        ones_row = consts.tile([1, 128], BF16)
        nc.vector.memset(ones_row, 1.0)
        ones_col = consts.tile([Q, 1], BF16)
        nc.vector.memset(ones_col, 1.0)

        for rep in range(reps):
          qn_all = qn_pool.tile([Q, BL, DPQ], BF16, tag="qn")
          nc.sync.dma_start(
              out=qn_all.rearrange("q b d -> q (b d)"),
              in_=qn_d.rearrange("q b d -> q (b d)"),
          )
          st = [dict() for _ in range(BL)]

          def S1(b):
            s = st[b]
            s["qn_b"] = qn_all[:, b, :]
            ct_all = ct_pool.tile([128, NCC, DPC], BF16, tag="ct")
            nc.sync.dma_start(
                out=ct_all.rearrange("p i d -> p (i d)"), in_=ctx_d[:, b, :]
            )
            s["ct"] = ct_all
            pts1 = pts_pool.tile([128, 256], BF16, tag="pts")
            for j in range(NDC):
                nc.tensor.transpose(
                    pts1[:, Q * j : Q * (j + 1)],
                    s["qn_b"][:, D + 4 + 128 * j : D + 4 + 128 * (j + 1)],
                    ident[:Q, :Q],
                )
            qnTw = sim_pool.tile([128, 2 * Q], BF16, tag="qnTw")
            nc.vector.tensor_copy(qnTw, pts1[:, : 2 * Q])
            s["qnTw"] = qnTw
            ptc = ptc_pool.tile([128, NDC, C], BF16, tag="ptc")
            for j in range(NDC):
                for i in range(NCC):
                    nc.tensor.transpose(
                        ptc[:, j, 128 * i : 128 * (i + 1)],
                        ct_all[:, i, 128 * j : 128 * (j + 1)],
                        ident,
                    )
            cT = ctxT_pool.tile([128, NDC, C], BF16, tag="ctxT")
            nc.vector.tensor_copy(cT, ptc)
            s["cT"] = cT

          def S2(b):
            s = st[b]
            psim = psim_pool.tile([Q, C], F32, tag="psim")
            for j in range(NDC):
                nc.tensor.matmul(
                    psim,
                    s["qnTw"][:, Q * j : Q * (j + 1)],
                    s["cT"][:, j, :],
                    start=(j == 0),
                    stop=(j == NDC - 1),
                )
            expT = sim_pool.tile([Q, C], BF16, tag="expT")
            nc.scalar.activation(
                expT, psim, EXP, bias=s["qn_b"][:, D : D + 1], scale=1.0
            )
            s["expT"] = expT
            sim_t = sim_pool.tile([Q, C], BF16, tag="simt")
            nc.scalar.add(sim_t, psim, s["qn_b"][:, D + 2 : D + 4].bitcast(F32))
            s["sim_t"] = sim_t

          def S3(b):
            s = st[b]
            ptm = pts_pool.tile([128, 256], BF16, tag="pts")
            for i in range(NCC):
                nc.tensor.transpose(
                    ptm[:, Q * i : Q * (i + 1)],
                    s["sim_t"][:, 128 * i : 128 * (i + 1)],
                    ident[:Q, :Q],
                )
            t_col = small_pool.tile([128, NCC], F32, tag="tcol")
            nc.vector.reduce_max(
                t_col, ptm.rearrange("p (i q) -> p i q", q=Q), axis=AX
            )
            sm2 = small_pool.tile([128, NCC], F32, tag="sm2")
            nc.vector.tensor_add(sm2, t_col, s["ct"][:, :, D + 1])
            e_col = small_pool.tile([128, NCC], BF16, tag="ecol")
            nc.scalar.activation(e_col, sm2, EXP)
            s["e_col"] = e_col

          def S4(b):
            s = st[b]
            psm = psmall_pool.tile([128, 262], F32, tag="psmall")
            s["psm"] = psm
            for i in range(NCC):
                nc.tensor.matmul(
                    psm[:1, 0 : D + 1],
                    s["e_col"][:, i : i + 1],
                    s["ct"][:, i, : D + 1],
                    start=(i == 0),
                    stop=(i == NCC - 1),
                    skip_group_check=True,
                )
            s_rec = small_pool.tile([1, 1], F32, tag="srec")
            nc.vector.reciprocal(s_rec, psm[:1, D : D + 1])
            q2c_row = small_pool.tile([1, D], BF16, tag="q2crow")
            nc.scalar.activation(q2c_row, psm[:1, :D], COPY, scale=s_rec)
            pbc = pbc_pool.tile([128, D], F32, tag="pbc")
            nc.tensor.matmul(pbc, ones_row, q2c_row, start=True, stop=True)
            pbc_sb = small_pool.tile([128, D], BF16, tag="pbcsb")
            nc.vector.tensor_copy(pbc_sb, pbc)
            g_all = g_pool.tile([128, NCC, GW], BF16, tag="gall")
            s["g_all"] = g_all
            g_view = g_d[:, b, :].rearrange("p (i m) -> p i m", i=NCC)
            s["g_view"] = g_view
            nc.vector.tensor_mul(
                g_all[:, :, 2 * D : 3 * D],
                s["ct"][:, :, :D],
                pbc_sb[:, None, :].broadcast_to([128, NCC, D]),
            )
            nc.sync.dma_start(
                out=g_view[:, :, 2 * D :], in_=g_all[:, :, 2 * D :]
            )

          def S5(b):
            s = st[b]
            g_all, psm = s["g_all"], s["psm"]
            pc2q = []
            for p in range(2):
                pcq = pc2q_pool.tile([128, 2, D], F32, tag="pc2q")
                for k in range(2):
                    i = 2 * p + k
                    lhsT = s["expT"][:, 128 * i : 128 * (i + 1)]
                    nc.tensor.matmul(
                        pcq[:, k, :], lhsT, s["qn_b"][:, :D],
                        start=True, stop=True, skip_group_check=True,
                    )
                    nc.tensor.matmul(
                        psm[:, 258 + i : 259 + i], lhsT, ones_col,
                        start=True, stop=True, skip_group_check=True,
                    )
                pc2q.append(pcq)
            den_r = small_pool.tile([128, NCC], F32, tag="denr")
            nc.vector.reciprocal(den_r, psm[:, 258:262])
            for i in range(2):
                nc.scalar.activation(
                    g_all[:, i, 0:D], pc2q[0][:, i, :], COPY,
                    scale=den_r[:, i : i + 1],
                )
            nc.vector.tensor_mul(
                g_all[:, 2:4, 0:D],
                pc2q[1],
                den_r[:, 2:4, None].broadcast_to([128, 2, D]),
            )
            nc.vector.tensor_mul(
                g_all[:, 2:, D : 2 * D], s["ct"][:, 2:, :D], g_all[:, 2:, 0:D]
            )
            g3eng = nc.gpsimd if KNOBS["g3p0"] == "pool" else nc.vector
            g3eng.tensor_mul(
                g_all[:, :2, D : 2 * D], s["ct"][:, :2, :D], g_all[:, :2, 0:D]
            )
            nc.sync.dma_start(
                out=s["g_view"][:, :, : 2 * D], in_=g_all[:, :, : 2 * D]
            )

          stages = [S1, S2, S3, S4, S5]
          for i in range(BL + len(stages) - 1):
              # oldest batch first: engines drain nearly-done work before
              # starting new batches (strict-FIFO friendliness)
              for k in range(len(stages) - 1, -1, -1):
                  if 0 <= i - k < BL:
                      stages[k](i - k)


def _emit_dmaonly(tc, ctx_d, qn_d, g_d, reps=1):
    """Experiment: just the DMA traffic of the real kernel, no compute."""
    nc = tc.nc
    with (
        tc.tile_pool(name="ct", bufs=4) as ct_pool,
        tc.tile_pool(name="qn", bufs=2) as qn_pool,
        tc.tile_pool(name="gout", bufs=3) as g_pool,
    ):
        for rep in range(reps):
            qn_all = qn_pool.tile([Q, BL, DPQ], BF16, tag="qn")
            nc.sync.dma_start(out=qn_all, in_=qn_d[:, :, :])
            for b in range(BL):
                ct_all = ct_pool.tile([128, NCC, DPC], BF16, tag="ct")
                nc.sync.dma_start(
                    out=ct_all,
                    in_=ctx_d[:, b, :].rearrange("p (i d) -> p i d", i=NCC),
                )
                g_all = g_pool.tile([128, NCC, GW], BF16, tag="gall")
                nc.vector.memset(g_all[:, :, 0:2], 1.0)
                nc.sync.dma_start(
                    out=g_d[:, b, :].rearrange("p (i m) -> p i m", i=NCC),
                    in_=g_all,
                )




def _emit_probe(tc, ctx_d, qn_d, g_d, reps=1):
    """Per-op rate probe: 64x one op per rep, no DMA in the loop."""
    nc = tc.nc
    op = KNOBS.get("probe_op", "act_copy")
    with (
        tc.tile_pool(name="src", bufs=1) as src_pool,
        tc.tile_pool(name="dst", bufs=2) as dst_pool,
    ):
        a = src_pool.tile([128, 4, D], BF16)
        nc.vector.memset(a, 1.0)
        b = src_pool.tile([128, 4, D], BF16)
        nc.vector.memset(b, 0.5)
        af = src_pool.tile([128, 4, D], F32)
        nc.vector.memset(af, 1.0)
        scal = src_pool.tile([128, 1], F32)
        nc.vector.memset(scal, 0.25)
        row = src_pool.tile([128, D], BF16)
        nc.vector.memset(row, 2.0)
        out_t = src_pool.tile([128, 16], BF16)
        for rep in range(reps):
            for k in range(64):
                o = dst_pool.tile([128, 4, D], BF16, tag="o")
                if op == "act_copy":
                    nc.scalar.activation(
                        o[:, 0, :], a[:, k % 4, :], COPY, scale=scal
                    )
                elif op == "act_exp":
                    nc.scalar.activation(
                        o.rearrange("p i d -> p (i d)")[:Q, :C],
                        a.rearrange("p i d -> p (i d)")[:Q, :C], EXP,
                    )
                elif op == "dve_mul_pair":
                    nc.vector.tensor_mul(o[:, :2, :], a[:, :2, :], b[:, :2, :])
                elif op == "dve_mul_bcast":
                    nc.vector.tensor_mul(
                        o, a, row[:, None, :].broadcast_to([128, 4, D])
                    )
                elif op == "dve_copy_1024":
                    nc.vector.tensor_copy(
                        o.rearrange("p i d -> p (i d)"),
                        a.rearrange("p i d -> p (i d)"),
                    )
                elif op == "dve_mul_f32":
                    nc.vector.tensor_mul(o[:, :2, :], af[:, :2, :], b[:, :2, :])
                elif op == "pool_mul_pair":
                    nc.gpsimd.tensor_mul(o[:, :2, :], a[:, :2, :], b[:, :2, :])
                else:
                    raise ValueError(op)
            # keep a data dependency so reps chain
            nc.vector.tensor_copy(out_t[:, 0:4], o[:, 0, 0:4])
        nc.sync.dma_start(out=g_d[:, 0, 0:16], in_=out_t)


def build_module(compile=True, reps=1, variant="full"):
    nc = bacc.Bacc(trn_type="TRN2")
    ctx_d = nc.dram_tensor("context", [128, BL, NCC * DPC], BF16, kind="ExternalInput")
    qn_d = nc.dram_tensor("question", [Q, BL, DPQ], BF16, kind="ExternalInput")
    g_d = nc.dram_tensor("g", [128, BL, NCC * GW], BF16, kind="ExternalOutput")
    emit = {"full": _emit, "dmaonly": _emit_dmaonly, "probe": _emit_probe}[variant]
    with tile.TileContext(nc) as tc:
        emit(tc, ctx_d, qn_d, g_d, reps=reps)
    if compile:
        nc.compile()
    return nc


_NC_CACHE = None


def _get_module():
    global _NC_CACHE
    if _NC_CACHE is None:
        _NC_CACHE = build_module()
    return _NC_CACHE


def make_in_maps(context, question, context_mask, question_mask, w):
    context = np.asarray(context, dtype=np.float32)
    question = np.asarray(question, dtype=np.float32)
    w = np.asarray(w, dtype=np.float32)
    w_c, w_q, w_m = w[:D], w[D : 2 * D], w[2 * D :]
    cmadd = (np.asarray(context_mask, dtype=np.float32) - 1.0) * BIG
    qmadd = (np.asarray(question_mask, dtype=np.float32) - 1.0) * BIG
    cq = context @ w_c  # [B, C]
    qq = question @ w_q  # [B, Q]

    ctx_p = np.zeros((B, C, DPC), dtype=np.float32)
    ctx_p[:, :, :D] = context
    ctx_p[:, :, D] = 1.0
    ctx_p[:, :, D + 1] = cq + cmadd
    ctx_p = ctx_p.astype(NPBF16)
    # partition-major: [128, B, NCC*DPC]
    ctx_p = ctx_p.reshape(B, NCC, 128, DPC).transpose(2, 0, 1, 3)
    ctx_p = ctx_p.reshape(128, B, NCC * DPC)

    qn_p = np.zeros((B, Q, DPQ), dtype=np.float32)
    qn_p[:, :, :D] = question
    qn_p[:, :, D] = qq + qmadd
    qn_p[:, :, D + 4 :] = question * w_m
    qn_p = qn_p.astype(NPBF16)
    # qq as raw f32 occupying two bf16 slots (4-byte aligned at col D+2)
    qn_p.view(np.uint16)[:, :, D + 2 : D + 4] = (
        qq.astype(np.float32).view(np.uint32).astype(np.uint32)[..., None]
        .view(np.uint16).reshape(B, Q, 2)
    )
    qn_p = qn_p.transpose(1, 0, 2)  # [Q, B, DPQ]

    in_maps = []
    for k in range(N_CORES):
        sl = slice(k * BL, (k + 1) * BL)
        in_maps.append(
            {
                "context": np.ascontiguousarray(ctx_p[:, sl]),
                "question": np.ascontiguousarray(qn_p[:, sl]),
            }
        )
    return in_maps


def kernel(context, question, context_mask, question_mask, w):
    nc = _get_module()
    in_maps = make_in_maps(context, question, context_mask, question_mask, w)
    res = run_bass_kernel_spmd(nc, in_maps, list(range(N_CORES)))
    out = np.empty((B, C, 4 * D), dtype=np.float32)
    out[:, :, :D] = np.asarray(context, dtype=np.float32)
    for k in range(N_CORES):
        gk = np.asarray(res.results[k]["g"])  # [128, BL, NCC*GW] bf16
        gk = gk.reshape(128, BL, NCC, GW).transpose(1, 2, 0, 3).reshape(BL, C, GW)
        out[k * BL : (k + 1) * BL, :, D:] = gk.astype(np.float32)
    return out
